# revision 11
# baseline (speedup 1.0000x reference)
"""Bass/Trainium2 kernel for 9x9 bilateral denoising (edge-preserving blend).

Reference computation (per pixel, 9x9 neighborhood, C=3):
    dist  = sum_c (p_c - x_c)^2
    w     = exp(-50 * dist) * gauss2d(sigma=3)
    out   = clip(0.8 * x + 0.2 * (sum w*p / sum w), 0, 1)

Sharding: pure data parallel. 8 cores = 2 images x 4 horizontal bands of 128
rows. Host pre-pads each image by 4 (reflect) and ships planar fp16
[3,137,520] band tensors (rows 128v..128v+135 of the padded image), so each
of the 81 taps is a row-shift (9 preloaded tiles, one per vertical offset) x
a free-dim column slice. A column-shifted copy of each tile is derived
on-device (SBUF->SBUF DMA) so odd column offsets keep fp16 operands 4-byte
aligned for the DVE 2x mode.

Device program: taps are batched per (row-offset, column-parity) into 18
groups of J=5 (even j) or J=4 (odd j) taps using overlapping-window access
patterns [jj, c, w]:
    diff (DVE fp16 2x) -> Square (ACT) -> dist adds (DVE + GPSIMD) ->
    exp(-50*d) (ACT) -> w*p (DVE fp16 2x, broadcast) ->
    per-tap scaled-identity matmuls (TensorE) accumulating
    [num_R num_G num_B den] into 4 PSUM banks; the gaussian g_ij rides in
    the matmul's stationary operand (g*I), so no per-tap bias is needed.
Finish: reciprocal, num*r, fused blend (0.8x + 0.2t = 0.8*(x + 0.25t)),
fused scale+clip, DMA out.
"""

import math
import sys

for _p in ("/opt/trn_rl_repo",):
    if _p not in sys.path:
        sys.path.insert(0, _p)

import numpy as np

import bass_rust
import concourse.mybir as mybir
from concourse import bacc, bass_utils
from concourse.masks import make_identity
from concourse.tile import TileContext

# ---------------------------------------------------------------- constants
N_CORES = 8
B, H, W, C = 2, 512, 512, 3
KER = 9
PAD = 4
BAND = 128  # rows per core
PW = W + 2 * PAD  # padded width 520
EXP_SCALE = -50.0  # -0.5 / sigma_color^2
EDGE = 0.8

F16 = mybir.dt.float16
F32 = mybir.dt.float32


def _gauss2d():
    ax = np.arange(KER, dtype=np.float64) - (KER - 1) / 2.0
    g = np.exp(-0.5 * (ax / 3.0) ** 2)
    g = g / g.sum()
    return np.outer(g, g)


GAUSS2D = _gauss2d()


def _wap(base, offset, dims):
    """Arbitrary windowed AP on a tile: keep partition dim, set free dims."""
    b = base.copy()
    b.ap = bass_rust.VecI64Pair(
        [tuple(base.ap[0])] + [(int(s), int(c)) for s, c in dims]
    )
    b.offset = int(base.offset + offset)
    return b


# ---------------------------------------------------------------- program
def build_program(n_reps=1, cfg=None):
    cfg = dict(cfg or {})
    bcast_mul = cfg.get("bcast_mul", True)
    sq_dtype = F16 if cfg.get("sq16", True) else F32
    add1_gps = cfg.get("add1_gps", "alt")
    add2_gps = cfg.get("add2_gps", True)
    cfg.setdefault("center_bcast", True)
    bufs = cfg.get("bufs", {"diff": 3, "prod": 2})

    nc = bacc.Bacc(
        "TRN2", target_bir_lowering=False, debug=False, num_devices=N_CORES
    )

    pe_d = nc.dram_tensor("pe", [C, BAND + KER, PW], F16, kind="ExternalInput")
    x32_d = nc.dram_tensor("x32", [BAND, C * W], F32, kind="ExternalInput")
    out_d = nc.dram_tensor("out", [BAND, C * W], F32, kind="ExternalOutput")

    AluOp = mybir.AluOpType
    ActFn = mybir.ActivationFunctionType

    with TileContext(nc) as tc:
        with (
            tc.tile_pool(name="persist", bufs=1) as pp,
            tc.tile_pool(name="pipe", bufs=1) as pipe,
            tc.tile_pool(name="psum", bufs=1, space="PSUM") as psp,
        ):
            # ---- persistent loads: 9 row-shifted tiles + derived odd copies
            pe_rows = pe_d.ap().rearrange("c r w -> r c w")  # [137, 3, 520]
            te_map, to_map = {}, {}
            for i in [4, 0, 1, 2, 3, 5, 6, 7, 8]:
                t = pp.tile([BAND, C * PW], F16, tag=f"te{i}")
                nc.sync.dma_start(
                    out=t[:].rearrange("p (c w) -> p c w", c=C),
                    in_=pe_rows[i : i + BAND],
                )
                te_map[i] = t
                o = pp.tile([BAND, C * PW], F16, tag=f"to{i}")
                # odd-alignment copy: o[:, k] = t[:, k+1]
                nc.sync.dma_start(
                    out=o[:, : C * PW - 1], in_=t[:, 1 : C * PW]
                )
                to_map[i] = o
            te = [te_map[i] for i in range(KER)]
            to = [to_map[i] for i in range(KER)]

            x32 = pp.tile([BAND, C * W], F32, tag="x32")
            nc.sync.dma_start(out=x32[:], in_=x32_d.ap())

            # center patch operand for batched diff: either an explicit
            # 5x-replicated tile (step-1 reads) or a jj-step-0 broadcast view
            center_bcast = cfg.get("center_bcast", False)
            if not center_bcast:
                xrep = pp.tile([BAND, 5 * C * W], F16, tag="xrep")
                xc16 = _wap(te[4][:], PAD, [(PW, C), (1, W)])
                for r in range(5):
                    nc.vector.tensor_copy(
                        out=_wap(xrep[:], r * C * W, [(W, C), (1, W)]), in_=xc16
                    )

            # scaled identities: one per distinct gaussian weight (25)
            ident = pp.tile([128, 128], F16, tag="ident")
            make_identity(nc, ident[:])
            gident = {}
            for i in range(KER):
                for j in range(KER):
                    key = (min(i, KER - 1 - i), min(j, KER - 1 - j))
                    if key not in gident:
                        gt = pp.tile([128, 128], F16, tag=f"gid{key[0]}{key[1]}")
                        nc.vector.tensor_scalar_mul(
                            gt[:], ident[:], float(GAUSS2D[i, j])
                        )
                        gident[key] = gt

            scol = pp.tile([128, 1], F32, tag="scol")
            nc.gpsimd.memset(scol[:], EXP_SCALE)

            outt = pp.tile([BAND, C * W], F32, tag="outt")

            groups = [(i, par) for i in range(KER) for par in (0, 1)]

            for _rep in range(n_reps):
                ps = psp.tile([128, 4 * W], F32, tag="acc")  # 4 psum banks
                t_ctr = 0

                for gi, (i, par) in enumerate(groups):
                    J = 5 if par == 0 else 4
                    src = te[i] if par == 0 else to[i]

                    patches = _wap(src[:], 0, [(2, J), (PW, C), (1, W)])
                    if center_bcast:
                        xin = _wap(te[4][:], PAD, [(0, J), (PW, C), (1, W)])
                    else:
                        xin = _wap(xrep[:], 0, [(C * W, J), (W, C), (1, W)])

                    diff = pipe.tile(
                        [BAND, 5 * C * W], F16, tag=bufs.get("diff_tag", "diff"),
                        bufs=bufs.get("diff", 2),
                    )
                    nc.vector.tensor_tensor(
                        _wap(diff[:], 0, [(C * W, J), (W, C), (1, W)]),
                        patches,
                        xin,
                        AluOp.subtract,
                    )

                    if cfg.get("sq_inplace", True):
                        sq = diff
                    else:
                        sq = pipe.tile(
                            [BAND, 5 * C * W], sq_dtype, tag="sq",
                            bufs=bufs.get("sq", 1),
                        )
                    nc.scalar.activation(
                        sq[:, : J * C * W], diff[:, : J * C * W], ActFn.Square
                    )

                    d = pipe.tile(
                        [BAND, 5 * W], F32, tag="d", bufs=bufs.get("d", 2)
                    )
                    dv = _wap(d[:], 0, [(W, J), (1, W)])
                    s0 = _wap(sq[:], 0, [(C * W, J), (1, W)])
                    s1 = _wap(sq[:], W, [(C * W, J), (1, W)])
                    s2 = _wap(sq[:], 2 * W, [(C * W, J), (1, W)])
                    if add1_gps == "alt":
                        e1 = nc.gpsimd if gi % 2 else nc.vector
                    else:
                        e1 = nc.gpsimd if add1_gps else nc.vector
                    e2 = nc.gpsimd if add2_gps else nc.vector
                    e1.tensor_tensor(dv, s0, s1, AluOp.add)
                    e2.tensor_tensor(dv, dv, s2, AluOp.add)

                    w = pipe.tile(
                        [BAND, 5 * W], F16, tag="w", bufs=bufs.get("w", 2)
                    )
                    nc.scalar.activation(
                        w[:, : J * W],
                        d[:, : J * W],
                        ActFn.Exp,
                        scale=scol[:, 0:1],
                    )

                    prod = pipe.tile(
                        [BAND, 5 * C * W], F16, tag=bufs.get("prod_tag", "prod"),
                        bufs=bufs.get("prod", 2),
                    )
                    if bcast_mul:
                        nc.vector.tensor_tensor(
                            _wap(prod[:], 0, [(C * W, J), (W, C), (1, W)]),
                            patches,
                            _wap(w[:], 0, [(W, J), (0, C), (1, W)]),
                            AluOp.mult,
                        )
                    else:
                        for c in range(C):
                            nc.vector.tensor_tensor(
                                _wap(prod[:], c * W, [(C * W, J), (1, W)]),
                                _wap(src[:], c * PW, [(2, J), (1, W)]),
                                _wap(w[:], 0, [(W, J), (1, W)]),
                                AluOp.mult,
                            )

                    for jj in range(J):
                        j = 2 * jj + par
                        key = (min(i, KER - 1 - i), min(j, KER - 1 - j))
                        lhsT = gident[key]
                        first = t_ctr == 0
                        last = t_ctr == KER * KER - 1
                        for c in range(C):
                            nc.tensor.matmul(
                                ps[:, c * W : (c + 1) * W],
                                lhsT[:],
                                prod[:, (jj * C + c) * W : (jj * C + c + 1) * W],
                                start=first,
                                stop=last,
                            )
                        nc.tensor.matmul(
                            ps[:, 3 * W : 4 * W],
                            lhsT[:],
                            w[:, jj * W : (jj + 1) * W],
                            start=first,
                            stop=last,
                        )
                        t_ctr += 1

                # ---- finish: out = clip(0.8*(x + 0.25*num/den), 0, 1)
                # (all terms >= 0, so the lower clip is a mathematical no-op;
                #  XLA's clamp(0, v, 1) agrees bit-for-bit for v >= 0)
                r = pipe.tile([BAND, W], F32, tag="recip")
                nc.vector.reciprocal(r[:], ps[:, 3 * W : 4 * W])
                t3 = pipe.tile([BAND, C * W], F32, tag="t3")
                for c in range(C):
                    nc.vector.tensor_mul(
                        t3[:, c * W : (c + 1) * W],
                        ps[:, c * W : (c + 1) * W],
                        r[:],
                    )
                nc.vector.scalar_tensor_tensor(
                    outt[:], t3[:], (1.0 - EDGE) / EDGE, x32[:],
                    AluOp.mult, AluOp.add,
                )
                nc.vector.tensor_scalar(
                    outt[:], outt[:], EDGE, 1.0, AluOp.mult, AluOp.min
                )

            nc.sync.dma_start(out=out_d.ap(), in_=outt[:])

    nc.compile()
    return nc


# ---------------------------------------------------------------- host side
def prep_inputs(images):
    """images [2,512,512,3] fp32 -> list of 8 per-core input dicts."""
    images = np.asarray(images, dtype=np.float32)
    in_maps = []
    for b in range(B):
        xpad = np.pad(images[b], ((PAD, PAD), (PAD, PAD), (0, 0)), mode="reflect")
        xp = np.ascontiguousarray(xpad.transpose(2, 0, 1))  # [3, 520, 520]
        for v in range(H // BAND):
            band = xp[:, BAND * v : BAND * v + BAND + 2 * PAD, :]  # [3,136,520]
            pe = np.zeros((C, BAND + KER, PW), np.float16)
            pe[:, : BAND + 2 * PAD, :] = band.astype(np.float16)
            x32 = band[:, PAD : PAD + BAND, PAD : PAD + W]  # [3,128,512]
            x32 = np.ascontiguousarray(
                x32.transpose(1, 0, 2).reshape(BAND, C * W), dtype=np.float32
            )
            in_maps.append({"pe": pe, "x32": x32})
    return in_maps


def assemble_output(results):
    """8 per-core {'out': [128, 1536]} -> [2,512,512,3] fp32."""
    full = np.empty((B, H, W, C), np.float32)
    cc = 0
    for b in range(B):
        for v in range(H // BAND):
            band = results[cc]["out"].reshape(BAND, C, W).transpose(0, 2, 1)
            full[b, BAND * v : BAND * (v + 1)] = band
            cc += 1
    return full


_NC_CACHE = {}


def get_program(n_reps=1, cfg=None):
    key = (n_reps, str(sorted((cfg or {}).items(), key=str)))
    if key not in _NC_CACHE:
        _NC_CACHE[key] = build_program(n_reps, cfg)
    return _NC_CACHE[key]


def run_program(nc, in_maps):
    res = bass_utils.run_bass_kernel_spmd(nc, in_maps, list(range(N_CORES)))
    return res.results


def kernel(images):
    nc = get_program(1, None)
    in_maps = prep_inputs(images)
    results = run_program(nc, in_maps)
    return assemble_output(results)


if __name__ == "__main__":
    rng = np.random.default_rng(0)
    imgs = rng.random((B, H, W, C), dtype=np.float32)
    out = kernel(imgs)
    print("out", out.shape, out.dtype, float(out.min()), float(out.max()))


# revision 19
# speedup vs baseline: 1.5174x; 1.5174x over previous
"""Bass/Trainium2 kernel for 9x9 bilateral denoising (edge-preserving blend).

Reference computation (per pixel, 9x9 neighborhood, C=3):
    dist  = sum_c (p_c - x_c)^2
    w     = exp(-50 * dist) * gauss2d(sigma=3)
    out   = clip(0.8 * x + 0.2 * (sum w*p / sum w), 0, 1)

Sharding: pure data parallel. 8 cores = 2 images x 4 horizontal bands of 128
rows. Host pre-pads each image by 4 (reflect) and ships planar fp16
[3,137,520] band tensors, so every tap is a row-shift (9 preloaded SBUF
tiles, one per vertical offset) x a free-dim column slice. A column-shifted
copy of each tile is derived on-device (SBUF->SBUF DMA) so odd column
offsets keep fp16 operands 4-byte aligned for the DVE 2x mode.

Taps are batched per (row-offset i, column-parity) into 18 groups of J=5
(even j) or J=4 (odd j) taps using overlapping-window access patterns
[jj, c, w]:
    diff (DVE fp16 2x) -> Square in-place (ACT) -> dist adds (DVE/GPSIMD
    alternating) -> exp(-50*d) (ACT) -> w*patch (DVE fp16 2x, broadcast) ->
    per-tap scaled-identity matmuls (TensorE) accumulating
    [num_R num_G num_B den] into 4 PSUM banks; the gaussian g_ij rides in
    the matmul's stationary operand (g*I), so no per-tap bias op is needed.

Mirror mode (default): dist(q, q+delta) == dist(q+delta, q), so the exp
planes of row-groups i in {5..8} are not recomputed: they are shifted views
of the planes of groups {3..0} (computed on the extended 520-wide domain),
assembled by one partition+column-shifted SBUF->SBUF DMA plus a "sliver"
for the bottom (i-4) rows, which is computed for all mirrored groups at
once in two partition-packed op sets. Row i=4 mirrors within its own group
(pure column-shifted views, no DMA). This halves diff/square/dist/exp work.

Finish: reciprocal, num*r, fused blend (0.8x + 0.2t = 0.8*(x + 0.25t)),
fused scale+clip (values are >= 0 so the lower clip is a no-op), DMA out.
"""

import sys

for _p in ("/opt/trn_rl_repo",):
    if _p not in sys.path:
        sys.path.insert(0, _p)

import numpy as np

import bass_rust
import concourse.mybir as mybir
from concourse import bacc, bass_utils
from concourse.masks import make_identity
from concourse.tile import TileContext

# ---------------------------------------------------------------- constants
N_CORES = 8
B, H, W, C = 2, 512, 512, 3
KER = 9
PAD = 4
BAND = 128  # rows per core
PW = W + 2 * PAD  # padded width 520
PS = PW + 8  # te tile plane stride (4 pad cols each side) = 528
EXP_SCALE = -50.0  # -0.5 / sigma_color^2
EDGE = 0.8

F16 = mybir.dt.float16
F32 = mybir.dt.float32


def _gauss2d():
    ax = np.arange(KER, dtype=np.float64) - (KER - 1) / 2.0
    g = np.exp(-0.5 * (ax / 3.0) ** 2)
    g = g / g.sum()
    return np.outer(g, g)


GAUSS2D = _gauss2d()


def _wap(base, offset, dims):
    """Arbitrary windowed AP on a tile: keep partition dim, set free dims."""
    b = base.copy()
    b.ap = bass_rust.VecI64Pair(
        [tuple(base.ap[0])] + [(int(s), int(c)) for s, c in dims]
    )
    b.offset = int(base.offset + offset)
    return b


# ---------------------------------------------------------------- program
def build_program(n_reps=1, cfg=None):
    cfg = dict(cfg or {})
    mirror = cfg.get("mirror", True)
    add1_gps = cfg.get("add1_gps", True)
    add2_gps = cfg.get("add2_gps", True)
    bufs = cfg.get("bufs", {"wm": 3, "wsrc": 4})
    WW = PW if mirror else W  # front-end compute width
    FOF = 0 if mirror else PAD  # front-end patch col offset

    nc = bacc.Bacc(
        "TRN2", target_bir_lowering=False, debug=False, num_devices=N_CORES
    )

    pe_d = nc.dram_tensor("pe", [C, BAND + KER, PW], F16, kind="ExternalInput")
    x32_d = nc.dram_tensor("x32", [BAND, C * W], F32, kind="ExternalInput")
    out_d = nc.dram_tensor("out", [BAND, C * W], F32, kind="ExternalOutput")

    AluOp = mybir.AluOpType
    ActFn = mybir.ActivationFunctionType

    # group order: mirror pairs adjacent so the source plane is consumed
    # right after it is produced; row 4 (self-mirrored) last
    if mirror:
        order = []
        for i in range(4):
            order += [(i, 0), (i, 1), (8 - i, 0), (8 - i, 1)]
        order += [(4, 0), (4, 1)]
        # packed sliver rows for mirrored groups (bottom di rows each)
        pack = {}
        off = [0, 32]  # even pack at partitions 0..9, odd at 32..41
        # (engine ops can only start at 32-aligned partitions)
        for i in range(5, 9):
            for par in (0, 1):
                pack[(i, par)] = off[par]
                off[par] += i - 4
    else:
        order = [(i, par) for i in range(KER) for par in (0, 1)]

    with TileContext(nc) as tc:
        with (
            tc.tile_pool(name="persist", bufs=1) as pp,
            tc.tile_pool(name="pipe", bufs=1) as pipe,
            tc.tile_pool(name="psum", bufs=1, space="PSUM") as psp,
        ):
            # ---- loads: 9 row-shifted tiles + derived odd-alignment copies
            pe_rows = pe_d.ap().rearrange("c r w -> r c w")  # [137, 3, 520]
            te_map, to_map = {}, {}
            load_order = (
                [4, 0, 8, 7, 6, 5, 1, 2, 3] if mirror
                else [4, 0, 1, 2, 3, 5, 6, 7, 8]
            )
            for i in load_order:
                t = pp.tile([BAND, C * PS], F16, tag=f"te{i}")
                # zero the 4-col pads flanking each 520-wide plane so the
                # windowed reads that graze them stay finite
                nc.vector.memset(_wap(t[:], 0, [(PS, C), (1, PAD)]), 0.0)
                nc.vector.memset(
                    _wap(t[:], PAD + PW, [(PS, C), (1, PAD)]), 0.0
                )
                nc.sync.dma_start(
                    out=_wap(t[:], PAD, [(PS, C), (1, PW)]),
                    in_=pe_rows[i : i + BAND],
                )
                te_map[i] = t
                o = pp.tile([BAND, C * PS], F16, tag=f"to{i}")
                nc.vector.memset(o[:, C * PS - 1 :], 0.0)
                nc.sync.dma_start(out=o[:, : C * PS - 1], in_=t[:, 1 : C * PS])
                to_map[i] = o
            te = [te_map[i] for i in range(KER)]
            to = [to_map[i] for i in range(KER)]

            x32 = pp.tile([BAND, C * W], F32, tag="x32")
            nc.sync.dma_start(out=x32[:], in_=x32_d.ap())

            # scaled identities: one per distinct gaussian weight (25)
            ident = pp.tile([128, 128], F16, tag="ident")
            make_identity(nc, ident[:])
            gident = {}
            for i in range(KER):
                for j in range(KER):
                    key = (min(i, KER - 1 - i), min(j, KER - 1 - j))
                    if key not in gident:
                        gt = pp.tile([128, 128], F16, tag=f"gid{key[0]}{key[1]}")
                        nc.vector.tensor_scalar_mul(
                            gt[:], ident[:], float(GAUSS2D[i, j])
                        )
                        gident[key] = gt

            scol = pp.tile([128, 1], F32, tag="scol")
            nc.gpsimd.memset(scol[:], EXP_SCALE)

            outt = pp.tile([BAND, C * W], F32, tag="outt")

            add_ctr = [0]

            def front_end(src_ap, cen_ap, J, wout_ap, p0=0):
                """diff -> square -> dist adds -> exp over [J, C, WW].

                Operates on partitions [p0, p0+n) of src_ap/wout_ap (packed
                sliver rows use a 32-aligned sub-range; groups use all 128).
                """
                np0 = src_ap.partition_size()
                diff = pipe.tile(
                    [BAND, 5 * C * WW], F16, tag="diff",
                    bufs=bufs.get("diff", 2),
                )
                da = diff[p0 : p0 + np0]
                dv = _wap(da, 0, [(C * WW, J), (WW, C), (1, WW)])
                nc.vector.tensor_tensor(dv, src_ap, cen_ap, AluOp.subtract)
                nc.scalar.activation(
                    da[:, : J * C * WW], da[:, : J * C * WW], ActFn.Square
                )
                d = pipe.tile(
                    [BAND, 5 * WW], F32, tag="d", bufs=bufs.get("d", 2)
                )
                ddv = _wap(d[p0 : p0 + np0], 0, [(WW, J), (1, WW)])
                s0 = _wap(da, 0, [(C * WW, J), (1, WW)])
                s1 = _wap(da, WW, [(C * WW, J), (1, WW)])
                s2 = _wap(da, 2 * WW, [(C * WW, J), (1, WW)])
                if add1_gps == "alt":
                    e1 = nc.gpsimd if add_ctr[0] % 2 else nc.vector
                    add_ctr[0] += 1
                else:
                    e1 = nc.gpsimd if add1_gps else nc.vector
                e2 = nc.gpsimd if add2_gps else nc.vector
                e1.tensor_tensor(ddv, s0, s1, AluOp.add)
                e2.tensor_tensor(ddv, ddv, s2, AluOp.add)
                nc.scalar.activation(
                    wout_ap,
                    d[p0 : p0 + np0][:, : J * WW],
                    ActFn.Exp,
                    scale=scol[p0 : p0 + np0, 0:1],
                )

            # ---- sliver pre-pass (mirror mode): bottom di rows of every
            # mirrored group, partition-packed, two op sets (even/odd parity)
            if mirror:
                sliver_src = pp.tile([BAND, C * PS], F16, tag="slsrc")
                sliver_c = pp.tile([BAND, C * PS], F16, tag="slcen")
                w_sliv = pp.tile([BAND, 5 * PW], F16, tag="wsliv")
                for (i, par), pb in sorted(pack.items()):
                    di = i - 4
                    src = te[i] if par == 0 else to[i]
                    nc.sync.dma_start(
                        out=sliver_src[pb : pb + di, :],
                        in_=src[BAND - di : BAND, :],
                    )
                    nc.sync.dma_start(
                        out=sliver_c[pb : pb + di, :],
                        in_=te[4][BAND - di : BAND, :],
                    )
                for par, (p0, p1) in ((0, (0, 10)), (1, (32, 42))):
                    J = 5 - par
                    front_end(
                        _wap(sliver_src[p0:p1], 0, [(2, J), (PS, C), (1, WW)]),
                        _wap(sliver_c[p0:p1], PAD, [(0, J), (PS, C), (1, WW)]),
                        J,
                        w_sliv[p0:p1][:, : J * WW],
                        p0=p0,
                    )

            groups_wsrc = {}  # (i, par) -> w tile (width-WW layout)

            for _rep in range(n_reps):
                ps = psp.tile([128, 4 * W], F32, tag="acc")  # 4 psum banks
                t_ctr = 0
                n_taps = KER * KER

                def tap_matmuls(i, j, prod_slice, w_slice):
                    nonlocal t_ctr
                    key = (min(i, KER - 1 - i), min(j, KER - 1 - j))
                    lhsT = gident[key]
                    first = t_ctr == 0
                    last = t_ctr == n_taps - 1
                    for c in range(C):
                        nc.tensor.matmul(
                            ps[:, c * W : (c + 1) * W],
                            lhsT[:],
                            prod_slice(c),
                            start=first,
                            stop=last,
                        )
                    nc.tensor.matmul(
                        ps[:, 3 * W : 4 * W], lhsT[:], w_slice,
                        start=first, stop=last,
                    )
                    t_ctr += 1

                for i, par in order:
                    mirrored = mirror and i >= 5
                    row4 = mirror and i == 4
                    J = 5 if par == 0 else 4
                    src = te[i] if par == 0 else to[i]
                    # patches over output cols (for prod): tile col = c+j+4
                    # (odd parity reads the shifted copy -> same offsets)
                    patches_out = _wap(src[:], PAD, [(2, J), (PS, C), (1, W)])

                    if not mirrored and not row4:
                        wsrc = pipe.tile(
                            [BAND, 5 * WW], F16, tag="wsrc",
                            bufs=bufs.get("wsrc", 3),
                        )
                        front_end(
                            _wap(src[:], FOF, [(2, J), (PS, C), (1, WW)]),
                            _wap(te[4][:], FOF + PAD, [(0, J), (PS, C), (1, WW)]),
                            J,
                            wsrc[:, : J * WW],
                        )
                        groups_wsrc[(i, par)] = wsrc
                        wof = PAD if mirror else 0
                        prod = pipe.tile(
                            [BAND, 5 * C * W], F16, tag="prod",
                            bufs=bufs.get("prod", 2),
                        )
                        nc.vector.tensor_tensor(
                            _wap(prod[:], 0, [(C * W, J), (W, C), (1, W)]),
                            patches_out,
                            _wap(wsrc[:], wof, [(WW, J), (0, C), (1, W)]),
                            AluOp.mult,
                        )
                        for jj in range(J):
                            j = 2 * jj + par
                            tap_matmuls(
                                i, j,
                                lambda c, jj=jj: prod[
                                    :, (jj * C + c) * W : (jj * C + c + 1) * W
                                ],
                                _wap(wsrc[:], jj * WW + wof, [(1, W)]),
                            )

                    elif mirrored:
                        di = i - 4
                        msrc = groups_wsrc[(8 - i, par)]  # width-PW planes
                        pb = pack[(i, par)]
                        wm = pipe.tile(
                            [BAND, 5 * W], F16, tag="wm",
                            bufs=bufs.get("wm", 2),
                        )
                        # main shifted copy: wm[r, jj*W+c] =
                        #   msrc[r+di, (4-par-jj)*PW + c + 2jj + par]
                        mbase = (4 - par) * PW + par
                        nc.sync.dma_start(
                            out=_wap(wm[0 : BAND - di], 0, [(W, J), (1, W)]),
                            in_=_wap(msrc[di:BAND], mbase, [(2 - PW, J), (1, W)]),
                        )
                        # sliver scatter: bottom di rows
                        nc.sync.dma_start(
                            out=_wap(wm[BAND - di : BAND], 0, [(W, J), (1, W)]),
                            in_=_wap(w_sliv[pb : pb + di], PAD, [(PW, J), (1, W)]),
                        )
                        prod = pipe.tile(
                            [BAND, 5 * C * W], F16, tag="prod",
                            bufs=bufs.get("prod", 2),
                        )
                        nc.vector.tensor_tensor(
                            _wap(prod[:], 0, [(C * W, J), (W, C), (1, W)]),
                            patches_out,
                            _wap(wm[:], 0, [(W, J), (0, C), (1, W)]),
                            AluOp.mult,
                        )
                        for jj in range(J):
                            j = 2 * jj + par
                            tap_matmuls(
                                i, j,
                                lambda c, jj=jj: prod[
                                    :, (jj * C + c) * W : (jj * C + c + 1) * W
                                ],
                                wm[:, jj * W : (jj + 1) * W],
                            )

                    else:  # row4: compute low half, derive high half as views
                        Jc = 3 - par  # computed taps: j in {0,2,4} / {1,3}
                        wsrc = pipe.tile(
                            [BAND, 5 * WW], F16, tag="wsrc",
                            bufs=bufs.get("wsrc", 3),
                        )
                        front_end(
                            _wap(src[:], 0, [(2, Jc), (PS, C), (1, WW)]),
                            _wap(te[4][:], PAD, [(0, Jc), (PS, C), (1, WW)]),
                            Jc,
                            wsrc[:, : Jc * WW],
                        )
                        groups_wsrc[(i, par)] = wsrc
                        prod = pipe.tile(
                            [BAND, 5 * C * W], F16, tag="prod",
                            bufs=bufs.get("prod", 2),
                        )
                        # computed part
                        nc.vector.tensor_tensor(
                            _wap(prod[:], 0, [(C * W, Jc), (W, C), (1, W)]),
                            _wap(src[:], PAD, [(2, Jc), (PS, C), (1, W)]),
                            _wap(wsrc[:], PAD, [(PW, Jc), (0, C), (1, W)]),
                            AluOp.mult,
                        )
                        # derived part: taps j = 2jj+par for jj in [Jc, J);
                        # w view flat col = (4 - par - jj)*PW + c + 2jj + par
                        Jd = J - Jc
                        dbase = (4 - par - Jc) * PW + par + 2 * Jc
                        nc.vector.tensor_tensor(
                            _wap(prod[:], Jc * C * W,
                                 [(C * W, Jd), (W, C), (1, W)]),
                            _wap(src[:], PAD + 2 * Jc,
                                 [(2, Jd), (PS, C), (1, W)]),
                            _wap(wsrc[:], dbase, [(2 - PW, Jd), (0, C), (1, W)]),
                            AluOp.mult,
                        )
                        for jj in range(J):
                            j = 2 * jj + par
                            if jj < Jc:
                                wsl = _wap(wsrc[:], jj * WW + PAD, [(1, W)])
                            else:
                                wsl = _wap(
                                    wsrc[:], dbase + (jj - Jc) * (2 - PW),
                                    [(1, W)],
                                )
                            tap_matmuls(
                                i, j,
                                lambda c, jj=jj: prod[
                                    :, (jj * C + c) * W : (jj * C + c + 1) * W
                                ],
                                wsl,
                            )

                assert t_ctr == n_taps, t_ctr

                # ---- finish: out = clip(0.8*(x + 0.25*num/den), 0, 1)
                # finals reuse pipe slots: r fits in a "d" slot, t3 in a
                # "prod" slot (tag sizing = max over tiles with that tag)
                r = pipe.tile([BAND, W], F32, tag="d", bufs=bufs.get("d", 2))
                nc.vector.reciprocal(r[:], ps[:, 3 * W : 4 * W])
                t3 = pipe.tile(
                    [BAND, C * W], F32, tag="prod", bufs=bufs.get("prod", 2)
                )
                for c in range(C):
                    nc.vector.tensor_mul(
                        t3[:, c * W : (c + 1) * W],
                        ps[:, c * W : (c + 1) * W],
                        r[:],
                    )
                nc.vector.scalar_tensor_tensor(
                    outt[:], t3[:], (1.0 - EDGE) / EDGE, x32[:],
                    AluOp.mult, AluOp.add,
                )
                nc.vector.tensor_scalar(
                    outt[:], outt[:], EDGE, 1.0, AluOp.mult, AluOp.min
                )

            nc.sync.dma_start(out=out_d.ap(), in_=outt[:])

    nc.compile()
    return nc


# ---------------------------------------------------------------- host side
def prep_inputs(images):
    """images [2,512,512,3] fp32 -> list of 8 per-core input dicts."""
    images = np.asarray(images, dtype=np.float32)
    in_maps = []
    for b in range(B):
        xpad = np.pad(images[b], ((PAD, PAD), (PAD, PAD), (0, 0)), mode="reflect")
        xp = np.ascontiguousarray(xpad.transpose(2, 0, 1))  # [3, 520, 520]
        for v in range(H // BAND):
            band = xp[:, BAND * v : BAND * v + BAND + 2 * PAD, :]  # [3,136,520]
            pe = np.zeros((C, BAND + KER, PW), np.float16)
            pe[:, : BAND + 2 * PAD, :] = band.astype(np.float16)
            x32 = band[:, PAD : PAD + BAND, PAD : PAD + W]  # [3,128,512]
            x32 = np.ascontiguousarray(
                x32.transpose(1, 0, 2).reshape(BAND, C * W), dtype=np.float32
            )
            in_maps.append({"pe": pe, "x32": x32})
    return in_maps


def assemble_output(results):
    """8 per-core {'out': [128, 1536]} -> [2,512,512,3] fp32."""
    full = np.empty((B, H, W, C), np.float32)
    cc = 0
    for b in range(B):
        for v in range(H // BAND):
            band = results[cc]["out"].reshape(BAND, C, W).transpose(0, 2, 1)
            full[b, BAND * v : BAND * (v + 1)] = band
            cc += 1
    return full


_NC_CACHE = {}


def get_program(n_reps=1, cfg=None):
    key = (n_reps, str(sorted((cfg or {}).items(), key=str)))
    if key not in _NC_CACHE:
        _NC_CACHE[key] = build_program(n_reps, cfg)
    return _NC_CACHE[key]


def run_program(nc, in_maps):
    res = bass_utils.run_bass_kernel_spmd(nc, in_maps, list(range(N_CORES)))
    return res.results


def kernel(images):
    nc = get_program(1, None)
    in_maps = prep_inputs(images)
    results = run_program(nc, in_maps)
    return assemble_output(results)


if __name__ == "__main__":
    rng = np.random.default_rng(0)
    imgs = rng.random((B, H, W, C), dtype=np.float32)
    out = kernel(imgs)
    print("out", out.shape, out.dtype, float(out.min()), float(out.max()))
